# revision 14
# baseline (speedup 1.0000x reference)
"""Chamfer distance (weighted, fwd+bwd, mean reduction) on 8 TRN2 NeuronCores.

Math: for pred P[b] (N=8192 x 3) and target T[b] (M=8192 x 3),
  sq(n, m) = |p_n - t_m|^2 = -2 * (p_n . t_m - |p_n|^2/2 - |t_m|^2/2)
One augmented matmul produces out(n, m) = p.t - |p|^2/2 - |t|^2/2 - DELTA
(strictly < 0); then min_m sq = -2 * (max_m out + DELTA) (sqrt is monotone,
applied on host).

The matmul runs in fp16 at full PE rate with a hi/lo split-precision expansion
that recovers fp32-level accuracy:
  p.t = ph.th + pl.th + ph.tl   (pl.tl ~ 2^-22, dropped)
K = 3*3 + 2 + 2 = 13 contraction rows; PE cost is K-independent.

Engine budget (per core, 32M distance-matrix elements): the f32 PSUM ->
fp16 SBUF convert must run on ScalarE/VectorE at 1 elem/lane/cycle (TRN2
matmul cannot write 16-bit PSUM), and pairwise max runs only on VectorE
(fp16 tensor_tensor, 2 elem/lane/cycle).  DMA bandwidth (~330 GB/s) is a
third, independent resource, so tiles come in two flavors tuned so that
ScalarE, VectorE and DMA all finish together:
 - light tiles (24): convert + DMA the full 2MB fp16 stage to DRAM; the
   host does both the forward (row) and backward (column) reductions for
   these rows via a uint16-bits min (exact: stage values are negative).
 - heavy tiles (8, t%4==1): VectorE also does the backward running max
   (8-tile acc chain) and a 3-level forward fold tree; only the 256KB f3
   partials go to DRAM.
Every tile's 4 PSUM slabs are converted by a mix of ScalarE activations
and VectorE casts (42 casts total balances the engines), and heavy-tile
VectorE work is chopped into ~1.2-2.3us pieces drained through a budget
scheduler so neither engine ever idles behind a long in-order burst.

Sharding: batch b -> core pair (2b, 2b+1); each core takes half the pred rows
(4096) and all 8192 targets. Backward partial maxes are combined on host.
"""

import numpy as np

import concourse.bacc as bacc
import concourse.mybir as mybir
import concourse.tile as tile
from concourse.bass_utils import run_bass_kernel_spmd

B = 4
N = 8192  # pred points per batch
M = 8192  # target points per batch
D = 3
K = 13  # augmented contraction dim (split precision)
NH = N // 2  # pred rows per core
P = 128  # partitions
NT = NH // P  # pred tiles per core (32)
SLAB = 2048  # psum slab width (4 banks)
NSLAB = M // SLAB  # 4
MM = 512  # matmul free dim (1 psum bank of f32)
NG = 4  # PE row-group strips at partitions 0/32/64/96
TFIRST = SLAB  # operand prefix that gates the first slab's matmuls
PFIRST = P
N_CORES = 8
EPS = 1e-12
DELTA = 2.0**-10  # subtracted inside the matmul so every stage value is < 0

# on-chip fwd+bwd tiles; the last one sits at t=27 so the final tiles are
# pure convert+ship and the backward flush overlaps them
HEAVY = (1, 5, 9, 13, 17, 21, 25, 27)
LIGHT = tuple(t for t in range(NT) if t not in HEAVY)
# VectorE casts this many PSUM slabs per tile (42 total balances ScalarE)
N_VCAST = {t: (2 if t % 3 == 2 else 1) for t in range(NT)}

# measured per-op VectorE costs (ns) for the piece scheduler
_COST_CAST = 2290
_TILE_V_BUDGET = 5300  # target VectorE ns per tile slot

_cached_nc = None


def _build_nc():
    f32 = mybir.dt.float32
    f16 = mybir.dt.float16
    alu_max = mybir.AluOpType.max

    nc = bacc.Bacc("TRN2", target_bir_lowering=False, debug=False)
    # Operands arrive pre-replicated into the 4 row-group strips (rows
    # 32g..32g+12 hold the data, the rest are zero), split into a small
    # first chunk (gates the first matmuls) and the bulk.
    pfirst = nc.dram_tensor("pfirst", [P, PFIRST], f16, kind="ExternalInput")
    prest = nc.dram_tensor("prest", [P, NH - PFIRST], f16, kind="ExternalInput")
    tfirst = nc.dram_tensor("tfirst", [P, TFIRST], f16, kind="ExternalInput")
    trest = nc.dram_tensor("trest", [P, M - TFIRST], f16, kind="ExternalInput")
    # st_out[i] = full fp16 stage of the i-th light tile (host reduces)
    st_out = nc.dram_tensor("st_out", [len(LIGHT), P, M], f16, kind="ExternalOutput")
    # f3_out[i] = 1024-wide forward partials of the i-th heavy tile
    f3_out = nc.dram_tensor("f3_out", [len(HEAVY), P, M // 8], f16, kind="ExternalOutput")
    # bwd_out[p, m] = max over the heavy tiles' pred rows congruent to p
    bwd_out = nc.dram_tensor("bwd_out", [P, M], f16, kind="ExternalOutput")

    with tile.TileContext(nc) as tc:
        with (
            tc.tile_pool(name="const", bufs=1) as cpool,
            tc.tile_pool(name="stage", bufs=6) as spool,
            tc.tile_pool(name="accp", bufs=2) as apool,
            tc.tile_pool(name="fold", bufs=2) as zpool,
            tc.tile_pool(name="psum", bufs=2, space="PSUM") as ppool,
        ):
            taug_sb = cpool.tile([P, M], f16)
            paug_sb = cpool.tile([P, NH], f16)
            nc.sync.dma_start(paug_sb[:, :PFIRST], pfirst[:])
            nc.sync.dma_start(taug_sb[:, :TFIRST], tfirst[:])
            nc.sync.dma_start(taug_sb[:, TFIRST:], trest[:])
            nc.scalar.dma_start(paug_sb[:, PFIRST:], prest[:])

            acc = None
            li = 0
            hi = 0
            pending = []  # (cost_ns, emit_fn) heavy V-work pieces

            def flush(budget):
                while pending and budget > 0:
                    cost, emit = pending.pop(0)
                    emit()
                    budget -= cost

            for t in range(NT):
                heavy = t in HEAVY
                if heavy and acc is None:
                    st = apool.tile([P, M], f16, tag="acc")  # first heavy: st==acc
                else:
                    st = spool.tile([P, M], f16, tag="st")
                nvc = N_VCAST[t]
                for s in range(NSLAB):
                    ps = ppool.tile([P, SLAB], f32, tag="ps")
                    for j in range(SLAB // MM):
                        col = s * SLAB + j * MM
                        g = 32 * (j % NG)
                        nc.tensor.matmul(
                            ps[:, j * MM : (j + 1) * MM],
                            paug_sb[g : g + K, t * P : (t + 1) * P],
                            taug_sb[g : g + K, col : col + MM],
                            start=True,
                            stop=True,
                            tile_position=(g, 0),
                        )
                    dst = st[:, s * SLAB : (s + 1) * SLAB]
                    if s >= NSLAB - nvc:
                        nc.vector.tensor_copy(dst, ps[:])
                    else:
                        nc.scalar.copy(dst, ps[:])
                    # ship each light slab as soon as it is converted — keeps
                    # the DMA queue fed smoothly and leaves no tail backlog
                    if not heavy:
                        nc.sync.dma_start(st_out[li, :, s * SLAB : (s + 1) * SLAB], dst)
                if not heavy:
                    li += 1
                else:
                    na, pieces = _heavy_pieces(
                        nc, apool, zpool, st, acc, hi, f3_out, bwd_out
                    )
                    pending.extend(pieces)
                    acc = na
                    hi += 1
                flush(_TILE_V_BUDGET - nvc * _COST_CAST)
            flush(1 << 30)
    nc.compile()
    return nc


def _heavy_pieces(nc, apool, zpool, st, acc, hi, f3_out, bwd_out):
    """Deferred VectorE pieces for a heavy tile: the backward running-max
    TT (two halves) and the forward fold tree, each ~0.7-2.3us."""
    f16 = mybir.dt.float16
    alu_max = mybir.AluOpType.max
    last = hi == len(HEAVY) - 1
    H = M // 2
    if acc is None:
        na = st  # first heavy tile: converts already wrote the accumulator
    else:
        na = apool.tile([P, M], f16, tag="acc")

    pieces = []
    if acc is not None:
        def tt_half(h):
            def emit():
                sl = slice(h * H, (h + 1) * H)
                nc.vector.tensor_tensor(na[:, sl], acc[:, sl], st[:, sl], op=alu_max)
                if last:
                    nc.scalar.dma_start(bwd_out[:, sl], na[:, sl])
            return emit

        pieces += [(2320, tt_half(0)), (2320, tt_half(1))]

    f1 = zpool.tile([P, M // 2], f16, tag="f1")
    f2 = zpool.tile([P, M // 4], f16, tag="f2")
    f3 = zpool.tile([P, M // 8], f16, tag="f3")
    Q = M // 4

    def f1_half(h):
        def emit():
            nc.vector.tensor_tensor(
                f1[:, h * Q : (h + 1) * Q],
                st[:, h * Q : (h + 1) * Q],
                st[:, H + h * Q : H + (h + 1) * Q],
                op=alu_max,
            )
        return emit

    def f23():
        nc.vector.tensor_tensor(f2[:], f1[:, : M // 4], f1[:, M // 4 :], op=alu_max)
        nc.vector.tensor_tensor(f3[:], f2[:, : M // 8], f2[:, M // 8 :], op=alu_max)
        nc.sync.dma_start(f3_out[hi], f3[:])

    pieces += [(1260, f1_half(0)), (1260, f1_half(1)), (1950, f23)]
    return na, pieces


def _get_nc():
    global _cached_nc
    if _cached_nc is None:
        _cached_nc = _build_nc()
    return _cached_nc


def _split16(x):
    """x (f32) -> (hi, lo) fp16 pair with hi + lo ~= x."""
    hi = x.astype(np.float16)
    lo = (x - hi.astype(np.float32)).astype(np.float16)
    return hi, lo


def _replicate_strips(aug):
    """[K, X] -> [128, X] with the data at partition offsets 0/32/64/96."""
    out = np.zeros((P, aug.shape[1]), np.float16)
    for g in range(NG):
        out[32 * g : 32 * g + K] = aug
    return out


def _make_in_maps(pred, target):
    in_maps = []
    for c in range(N_CORES):
        b, h = divmod(c, 2)
        p = pred[b, h * NH : (h + 1) * NH]  # [4096, 3]
        t = target[b]  # [8192, 3]
        pn = -0.5 * (p * p).sum(-1, dtype=np.float32) - 0.5 * DELTA
        tn = -0.5 * (t * t).sum(-1, dtype=np.float32) - 0.5 * DELTA
        ph, pl = _split16(p.T)
        th, tl = _split16(t.T)
        pnh, pnl = _split16(pn)
        tnh, tnl = _split16(tn)
        paug = np.zeros((K, NH), np.float16)
        taug = np.zeros((K, M), np.float16)
        # p.t = ph.th + pl.th + ph.tl ; norms via ones-rows
        paug[0:3] = ph
        paug[3:6] = pl
        paug[6:9] = ph
        paug[9] = pnh
        paug[10] = pnl
        paug[11] = 1.0
        paug[12] = 1.0
        taug[0:3] = th
        taug[3:6] = th
        taug[6:9] = tl
        taug[9] = 1.0
        taug[10] = 1.0
        taug[11] = tnh
        taug[12] = tnl
        prep = _replicate_strips(paug)
        trep = _replicate_strips(taug)
        in_maps.append({
            "pfirst": np.ascontiguousarray(prep[:, :PFIRST]),
            "prest": np.ascontiguousarray(prep[:, PFIRST:]),
            "tfirst": np.ascontiguousarray(trep[:, :TFIRST]),
            "trest": np.ascontiguousarray(trep[:, TFIRST:]),
        })
    return in_maps


def _negmax_bits(u16, axis):
    """Float max of strictly-negative fp16 values stored as uint16 bits:
    more-negative floats have larger bit patterns, so float max == bits min."""
    return np.ascontiguousarray(u16.min(axis=axis)).view(np.float16)


def _reduce_outputs(results):
    total = 0.0
    for b in range(B):
        fwd_rows = []
        bwd_parts = []
        for h in range(2):
            r = results[2 * b + h]
            st = np.asarray(r["st_out"]).view(np.uint16)  # [24, 128, 8192]
            f3 = np.asarray(r["f3_out"]).view(np.uint16)  # [8, 128, 1024]
            bw = np.asarray(r["bwd_out"]).view(np.uint16)  # [128, 8192]
            # forward: per-row max for every pred row, in original tile order
            fwd_light = _negmax_bits(st, axis=2).astype(np.float64)  # [24, 128]
            fwd_heavy = _negmax_bits(f3, axis=2).astype(np.float64)  # [8, 128]
            fwd = np.empty((NT, P))
            fwd[list(LIGHT)] = fwd_light
            fwd[list(HEAVY)] = fwd_heavy
            fwd_rows.append(fwd.reshape(-1))  # row order n = t*128 + p
            # backward: light tiles' columns from the shipped stages,
            # heavy tiles' columns from the on-chip accumulator
            bl = st.min(axis=(0, 1))
            bh = bw.min(axis=0)
            bwd_parts.append(
                np.minimum(bl, bh).view(np.float16).astype(np.float64)
            )
        fwd_max = np.concatenate(fwd_rows)  # [8192]
        bwd_max = np.maximum(bwd_parts[0], bwd_parts[1])  # [8192]
        fwd_sq = np.maximum(-2.0 * (fwd_max + DELTA), EPS)
        bwd_sq = np.maximum(-2.0 * (bwd_max + DELTA), EPS)
        total += np.sqrt(fwd_sq).sum() + np.sqrt(bwd_sq).sum()
    return np.asarray(total / B, dtype=np.float32)


def kernel(pred, target):
    pred = np.ascontiguousarray(np.asarray(pred, dtype=np.float32))
    target = np.ascontiguousarray(np.asarray(target, dtype=np.float32))
    assert pred.shape == (B, N, D) and target.shape == (B, M, D)
    nc = _get_nc()
    in_maps = _make_in_maps(pred, target)
    res = run_bass_kernel_spmd(nc, in_maps, list(range(N_CORES)))
    return _reduce_outputs(res.results)


# revision 18
# speedup vs baseline: 1.0429x; 1.0429x over previous
"""Chamfer distance (weighted, fwd+bwd, mean reduction) on 8 TRN2 NeuronCores.

Math: for pred P[b] (N=8192 x 3) and target T[b] (M=8192 x 3),
  sq(n, m) = |p_n - t_m|^2 = -2 * (p_n . t_m - |p_n|^2/2 - |t_m|^2/2)
One augmented matmul produces out(n, m) = p.t - |p|^2/2 - |t|^2/2 - DELTA
(strictly < 0); then min_m sq = -2 * (max_m out + DELTA) (sqrt is monotone,
applied on host).

The matmul runs in fp16 at full PE rate with a hi/lo split-precision expansion
that recovers fp32-level accuracy:
  p.t = ph.th + pl.th + ph.tl   (pl.tl ~ 2^-22, dropped)
K = 3*3 + 2 + 2 = 13 contraction rows; PE cost is K-independent.

Engine budget (per core, 32M distance-matrix elements): the f32 PSUM ->
fp16 SBUF convert must run on ScalarE/VectorE at 1 elem/lane/cycle (TRN2
matmul cannot write 16-bit PSUM), and pairwise max runs only on VectorE
(fp16 tensor_tensor, 2 elem/lane/cycle).  DMA bandwidth (~330 GB/s) is a
third, independent resource, so tiles come in two flavors tuned so that
ScalarE, VectorE and DMA all finish together:
 - light tiles (24): convert + DMA the full 2MB fp16 stage to DRAM; the
   host does both the forward (row) and backward (column) reductions for
   these rows via a uint16-bits min (exact: stage values are negative).
 - heavy tiles (8, t%4==1): VectorE also does the backward running max
   (8-tile acc chain) and a 3-level forward fold tree; only the 256KB f3
   partials go to DRAM.
Every tile's 4 PSUM slabs are converted by a mix of ScalarE activations
and VectorE casts (42 casts total balances the engines), and heavy-tile
VectorE work is chopped into ~1.2-2.3us pieces drained through a budget
scheduler so neither engine ever idles behind a long in-order burst.

Sharding: batch b -> core pair (2b, 2b+1); each core takes half the pred rows
(4096) and all 8192 targets. Backward partial maxes are combined on host.
"""

import numpy as np

import concourse.bacc as bacc
import concourse.mybir as mybir
import concourse.tile as tile
from concourse.bass_utils import run_bass_kernel_spmd

B = 4
N = 8192  # pred points per batch
M = 8192  # target points per batch
D = 3
K = 13  # augmented contraction dim (split precision)
NH = N // 2  # pred rows per core
P = 128  # partitions
NT = NH // P  # pred tiles per core (32)
SLAB = 2048  # psum slab width (4 banks)
NSLAB = M // SLAB  # 4
MM = 512  # matmul free dim (1 psum bank of f32)
NG = 4  # PE row-group strips at partitions 0/32/64/96
TFIRST = SLAB  # operand prefix that gates the first slab's matmuls
PFIRST = P
N_CORES = 8
EPS = 1e-12
DELTA = 2.0**-10  # subtracted inside the matmul so every stage value is < 0

# on-chip fwd+bwd tiles; the last one sits at t=27 so the final tiles are
# pure convert+ship and the backward flush overlaps them
HEAVY = (1, 5, 9, 13, 17, 21, 25, 27)
LIGHT = tuple(t for t in range(NT) if t not in HEAVY)
# Uniform per-tile convert split: ScalarE activates slabs 0, 1 and the
# first XSPLIT columns of slab 2; VectorE casts the rest.  Keeps both
# engines at ~5.4us on every tile (variance would stall the PSUM chain).
XSPLIT = 1280

# per-tile VectorE budget left for heavy-work pieces (ns)
_TILE_V_BUDGET = 2200

_cached_nc = None


def _build_nc():
    f32 = mybir.dt.float32
    f16 = mybir.dt.float16
    alu_max = mybir.AluOpType.max

    nc = bacc.Bacc("TRN2", target_bir_lowering=False, debug=False)
    # Operands arrive pre-replicated into the 4 row-group strips (rows
    # 32g..32g+12 hold the data, the rest are zero), split into a small
    # first chunk (gates the first matmuls) and the bulk.
    pfirst = nc.dram_tensor("pfirst", [P, PFIRST], f16, kind="ExternalInput")
    prest = nc.dram_tensor("prest", [P, NH - PFIRST], f16, kind="ExternalInput")
    tfirst = nc.dram_tensor("tfirst", [P, TFIRST], f16, kind="ExternalInput")
    trest = nc.dram_tensor("trest", [P, M - TFIRST], f16, kind="ExternalInput")
    # st_out[i] = full fp16 stage of the i-th light tile (host reduces)
    st_out = nc.dram_tensor("st_out", [len(LIGHT), P, M], f16, kind="ExternalOutput")
    # f3_out[i] = 1024-wide forward partials of the i-th heavy tile
    f3_out = nc.dram_tensor("f3_out", [len(HEAVY), P, M // 8], f16, kind="ExternalOutput")
    # bwd_out[p, m] = max over the heavy tiles' pred rows congruent to p
    bwd_out = nc.dram_tensor("bwd_out", [P, M], f16, kind="ExternalOutput")

    with tile.TileContext(nc) as tc:
        with (
            tc.tile_pool(name="const", bufs=1) as cpool,
            tc.tile_pool(name="stage", bufs=6) as spool,
            tc.tile_pool(name="accp", bufs=2) as apool,
            tc.tile_pool(name="fold", bufs=2) as zpool,
            tc.tile_pool(name="psum", bufs=2, space="PSUM") as ppool,
        ):
            taug_sb = cpool.tile([P, M], f16)
            paug_sb = cpool.tile([P, NH], f16)
            nc.sync.dma_start(paug_sb[:, :PFIRST], pfirst[:])
            nc.sync.dma_start(taug_sb[:, :TFIRST], tfirst[:])
            nc.sync.dma_start(taug_sb[:, TFIRST:], trest[:])
            nc.scalar.dma_start(paug_sb[:, PFIRST:], prest[:])

            acc = None
            li = 0
            hi = 0
            pending = []  # (cost_ns, emit_fn) heavy V-work pieces

            def flush(budget):
                while pending and budget > 0:
                    cost, emit = pending.pop(0)
                    emit()
                    budget -= cost

            for t in range(NT):
                heavy = t in HEAVY
                if heavy and acc is None:
                    st = apool.tile([P, M], f16, tag="acc")  # first heavy: st==acc
                else:
                    st = spool.tile([P, M], f16, tag="st")
                for s in range(NSLAB):
                    ps = ppool.tile([P, SLAB], f32, tag="ps")
                    for j in range(SLAB // MM):
                        col = s * SLAB + j * MM
                        g = 32 * (j % NG)
                        nc.tensor.matmul(
                            ps[:, j * MM : (j + 1) * MM],
                            paug_sb[g : g + K, t * P : (t + 1) * P],
                            taug_sb[g : g + K, col : col + MM],
                            start=True,
                            stop=True,
                            tile_position=(g, 0),
                        )
                    dst = st[:, s * SLAB : (s + 1) * SLAB]
                    if s < 2:
                        nc.scalar.copy(dst, ps[:])
                    elif s == 2:
                        nc.scalar.copy(dst[:, :XSPLIT], ps[:, :XSPLIT])
                        nc.vector.tensor_copy(dst[:, XSPLIT:], ps[:, XSPLIT:])
                    else:
                        nc.vector.tensor_copy(dst, ps[:])
                    # ship light stages per-slab near the end so the tail
                    # overlaps the final converts (whole-tile DMAs otherwise:
                    # small strided transfers waste DMA bandwidth)
                    if not heavy and t >= NT - 2:
                        nc.sync.dma_start(st_out[li, :, s * SLAB : (s + 1) * SLAB], dst)
                if not heavy:
                    if t < NT - 2:
                        nc.sync.dma_start(st_out[li], st[:])
                    li += 1
                else:
                    na, pieces = _heavy_pieces(
                        nc, apool, zpool, st, acc, hi, f3_out, bwd_out
                    )
                    pending.extend(pieces)
                    acc = na
                    hi += 1
                flush(_TILE_V_BUDGET)
            flush(1 << 30)
    nc.compile()
    return nc


def _heavy_pieces(nc, apool, zpool, st, acc, hi, f3_out, bwd_out):
    """Deferred VectorE pieces for a heavy tile: the backward running-max
    TT (two halves) and the forward fold tree, each ~0.7-2.3us."""
    f16 = mybir.dt.float16
    alu_max = mybir.AluOpType.max
    last = hi == len(HEAVY) - 1
    H = M // 2
    if acc is None:
        na = st  # first heavy tile: converts already wrote the accumulator
    else:
        na = apool.tile([P, M], f16, tag="acc")

    pieces = []
    if acc is not None:
        def tt_half(h):
            def emit():
                sl = slice(h * H, (h + 1) * H)
                nc.vector.tensor_tensor(na[:, sl], acc[:, sl], st[:, sl], op=alu_max)
                if last:
                    nc.scalar.dma_start(bwd_out[:, sl], na[:, sl])
            return emit

        pieces += [(2320, tt_half(0)), (2320, tt_half(1))]

    f1 = zpool.tile([P, M // 2], f16, tag="f1")
    f2 = zpool.tile([P, M // 4], f16, tag="f2")
    f3 = zpool.tile([P, M // 8], f16, tag="f3")
    Q = M // 4

    def f1_half(h):
        def emit():
            nc.vector.tensor_tensor(
                f1[:, h * Q : (h + 1) * Q],
                st[:, h * Q : (h + 1) * Q],
                st[:, H + h * Q : H + (h + 1) * Q],
                op=alu_max,
            )
        return emit

    def f23():
        nc.vector.tensor_tensor(f2[:], f1[:, : M // 4], f1[:, M // 4 :], op=alu_max)
        nc.vector.tensor_tensor(f3[:], f2[:, : M // 8], f2[:, M // 8 :], op=alu_max)
        nc.sync.dma_start(f3_out[hi], f3[:])

    pieces += [(1260, f1_half(0)), (1260, f1_half(1)), (1950, f23)]
    return na, pieces


def _get_nc():
    global _cached_nc
    if _cached_nc is None:
        _cached_nc = _build_nc()
    return _cached_nc


def _split16(x):
    """x (f32) -> (hi, lo) fp16 pair with hi + lo ~= x."""
    hi = x.astype(np.float16)
    lo = (x - hi.astype(np.float32)).astype(np.float16)
    return hi, lo


def _replicate_strips(aug):
    """[K, X] -> [128, X] with the data at partition offsets 0/32/64/96."""
    out = np.zeros((P, aug.shape[1]), np.float16)
    for g in range(NG):
        out[32 * g : 32 * g + K] = aug
    return out


def _make_in_maps(pred, target):
    in_maps = []
    for c in range(N_CORES):
        b, h = divmod(c, 2)
        p = pred[b, h * NH : (h + 1) * NH]  # [4096, 3]
        t = target[b]  # [8192, 3]
        pn = -0.5 * (p * p).sum(-1, dtype=np.float32) - 0.5 * DELTA
        tn = -0.5 * (t * t).sum(-1, dtype=np.float32) - 0.5 * DELTA
        ph, pl = _split16(p.T)
        th, tl = _split16(t.T)
        pnh, pnl = _split16(pn)
        tnh, tnl = _split16(tn)
        paug = np.zeros((K, NH), np.float16)
        taug = np.zeros((K, M), np.float16)
        # p.t = ph.th + pl.th + ph.tl ; norms via ones-rows
        paug[0:3] = ph
        paug[3:6] = pl
        paug[6:9] = ph
        paug[9] = pnh
        paug[10] = pnl
        paug[11] = 1.0
        paug[12] = 1.0
        taug[0:3] = th
        taug[3:6] = th
        taug[6:9] = tl
        taug[9] = 1.0
        taug[10] = 1.0
        taug[11] = tnh
        taug[12] = tnl
        prep = _replicate_strips(paug)
        trep = _replicate_strips(taug)
        in_maps.append({
            "pfirst": np.ascontiguousarray(prep[:, :PFIRST]),
            "prest": np.ascontiguousarray(prep[:, PFIRST:]),
            "tfirst": np.ascontiguousarray(trep[:, :TFIRST]),
            "trest": np.ascontiguousarray(trep[:, TFIRST:]),
        })
    return in_maps


def _negmax_bits(u16, axis):
    """Float max of strictly-negative fp16 values stored as uint16 bits:
    more-negative floats have larger bit patterns, so float max == bits min."""
    return np.ascontiguousarray(u16.min(axis=axis)).view(np.float16)


def _reduce_outputs(results):
    total = 0.0
    for b in range(B):
        fwd_rows = []
        bwd_parts = []
        for h in range(2):
            r = results[2 * b + h]
            st = np.asarray(r["st_out"]).view(np.uint16)  # [24, 128, 8192]
            f3 = np.asarray(r["f3_out"]).view(np.uint16)  # [8, 128, 1024]
            bw = np.asarray(r["bwd_out"]).view(np.uint16)  # [128, 8192]
            # forward: per-row max for every pred row, in original tile order
            fwd_light = _negmax_bits(st, axis=2).astype(np.float64)  # [24, 128]
            fwd_heavy = _negmax_bits(f3, axis=2).astype(np.float64)  # [8, 128]
            fwd = np.empty((NT, P))
            fwd[list(LIGHT)] = fwd_light
            fwd[list(HEAVY)] = fwd_heavy
            fwd_rows.append(fwd.reshape(-1))  # row order n = t*128 + p
            # backward: light tiles' columns from the shipped stages,
            # heavy tiles' columns from the on-chip accumulator
            bl = st.min(axis=(0, 1))
            bh = bw.min(axis=0)
            bwd_parts.append(
                np.minimum(bl, bh).view(np.float16).astype(np.float64)
            )
        fwd_max = np.concatenate(fwd_rows)  # [8192]
        bwd_max = np.maximum(bwd_parts[0], bwd_parts[1])  # [8192]
        fwd_sq = np.maximum(-2.0 * (fwd_max + DELTA), EPS)
        bwd_sq = np.maximum(-2.0 * (bwd_max + DELTA), EPS)
        total += np.sqrt(fwd_sq).sum() + np.sqrt(bwd_sq).sum()
    return np.asarray(total / B, dtype=np.float32)


def kernel(pred, target):
    pred = np.ascontiguousarray(np.asarray(pred, dtype=np.float32))
    target = np.ascontiguousarray(np.asarray(target, dtype=np.float32))
    assert pred.shape == (B, N, D) and target.shape == (B, M, D)
    nc = _get_nc()
    in_maps = _make_in_maps(pred, target)
    res = run_bass_kernel_spmd(nc, in_maps, list(range(N_CORES)))
    return _reduce_outputs(res.results)


# revision 21
# speedup vs baseline: 1.1503x; 1.1030x over previous
"""Chamfer distance (weighted, fwd+bwd, mean reduction) on 8 TRN2 NeuronCores.

Math: for pred P[b] (N=8192 x 3) and target T[b] (M=8192 x 3),
  sq(n, m) = |p_n - t_m|^2 = -2 * (p_n . t_m - |p_n|^2/2 - |t_m|^2/2)
One augmented matmul produces out(n, m) = p.t - |p|^2/2 - |t|^2/2 - DELTA
(strictly < 0); then min_m sq = -2 * (max_m out + DELTA) (sqrt is monotone,
applied on host).

The matmul runs in fp16 at full PE rate with a hi/lo split-precision expansion
that recovers fp32-level accuracy:
  p.t = ph.th + pl.th + ph.tl   (pl.tl ~ 2^-22, dropped)
K = 3*3 + 2 + 2 = 13 contraction rows; PE cost is K-independent.

Engine budget (per core, 32M distance-matrix elements): the f32 PSUM ->
fp16 SBUF convert must run on ScalarE/VectorE at 1 elem/lane/cycle (TRN2
matmul cannot write 16-bit PSUM), and pairwise max runs only on VectorE
(fp16 tensor_tensor, 2 elem/lane/cycle).  DMA bandwidth (~330 GB/s) is a
third, independent resource, so tiles come in two flavors tuned so that
ScalarE, VectorE and DMA all finish together:
 - light tiles (24): convert + DMA the full 2MB fp16 stage to DRAM; the
   host does both the forward (row) and backward (column) reductions for
   these rows via a uint16-bits min (exact: stage values are negative).
 - heavy tiles (8, t%4==1): VectorE also does the backward running max
   (8-tile acc chain) and a 3-level forward fold tree; only the 256KB f3
   partials go to DRAM.
Every tile's 4 PSUM slabs are converted by a mix of ScalarE activations
and VectorE casts (42 casts total balances the engines), and heavy-tile
VectorE work is chopped into ~1.2-2.3us pieces drained through a budget
scheduler so neither engine ever idles behind a long in-order burst.

Sharding: batch b -> core pair (2b, 2b+1); each core takes half the pred rows
(4096) and all 8192 targets. Backward partial maxes are combined on host.
"""

import numpy as np

import concourse.bacc as bacc
import concourse.mybir as mybir
import concourse.tile as tile
from concourse.bass_utils import run_bass_kernel_spmd

B = 4
N = 8192  # pred points per batch
M = 8192  # target points per batch
D = 3
K = 13  # augmented contraction dim (split precision)
NH = N // 2  # pred rows per core
P = 128  # partitions
NT = NH // P  # pred tiles per core (32)
SLAB = 2048  # psum slab width (4 banks)
NSLAB = M // SLAB  # 4
MM = 512  # matmul free dim (1 psum bank of f32)
NG = 4  # PE row-group strips at partitions 0/32/64/96
TFIRST = SLAB  # operand prefix that gates the first slab's matmuls
PFIRST = P
N_CORES = 8
EPS = 1e-12
DELTA = 2.0**-10  # subtracted inside the matmul so every stage value is < 0

# on-chip fwd+bwd tiles; the last one sits at t=27 so the final tiles are
# pure convert+ship and the backward flush overlaps them
HEAVY = (1, 5, 9, 13, 17, 21, 24, 27)
LIGHT = tuple(t for t in range(NT) if t not in HEAVY)
# VectorE casts this many PSUM slabs per tile (44 total balances ScalarE)
N_VCAST = {t: (2 if (t % 3 == 2 or t in (0, 16)) else 1) for t in range(NT)}

# per-tile VectorE emission budget for heavy-work pieces (ns); generous so
# queued pieces can fill VectorE's PSUM-chain stall windows
_TILE_V_BUDGET = 3600

_cached_nc = None


def _build_nc():
    f32 = mybir.dt.float32
    f16 = mybir.dt.float16
    alu_max = mybir.AluOpType.max

    nc = bacc.Bacc("TRN2", target_bir_lowering=False, debug=False)
    # Operands arrive pre-replicated into the 4 row-group strips (rows
    # 32g..32g+12 hold the data, the rest are zero), split into a small
    # first chunk (gates the first matmuls) and the bulk.
    pfirst = nc.dram_tensor("pfirst", [P, PFIRST], f16, kind="ExternalInput")
    prest = nc.dram_tensor("prest", [P, NH - PFIRST], f16, kind="ExternalInput")
    tfirst = nc.dram_tensor("tfirst", [P, TFIRST], f16, kind="ExternalInput")
    trest = nc.dram_tensor("trest", [P, M - TFIRST], f16, kind="ExternalInput")
    # st_out[i] = full fp16 stage of the i-th light tile (host reduces)
    st_out = nc.dram_tensor("st_out", [len(LIGHT), P, M], f16, kind="ExternalOutput")
    # f3_out[i] = 1024-wide forward partials of the i-th heavy tile
    f3_out = nc.dram_tensor("f3_out", [len(HEAVY), P, M // 8], f16, kind="ExternalOutput")
    # bwd_out[p, m] = max over the heavy tiles' pred rows congruent to p
    bwd_out = nc.dram_tensor("bwd_out", [P, M], f16, kind="ExternalOutput")

    with tile.TileContext(nc) as tc:
        with (
            tc.tile_pool(name="const", bufs=1) as cpool,
            tc.tile_pool(name="stage", bufs=6) as spool,
            tc.tile_pool(name="accp", bufs=2) as apool,
            tc.tile_pool(name="fold", bufs=2) as zpool,
            tc.tile_pool(name="psum", bufs=2, space="PSUM") as ppool,
        ):
            taug_sb = cpool.tile([P, M], f16)
            paug_sb = cpool.tile([P, NH], f16)
            nc.sync.dma_start(paug_sb[:, :PFIRST], pfirst[:])
            nc.sync.dma_start(taug_sb[:, :TFIRST], tfirst[:])
            nc.sync.dma_start(taug_sb[:, TFIRST:], trest[:])
            nc.scalar.dma_start(paug_sb[:, PFIRST:], prest[:])

            acc = None
            li = 0
            hi = 0
            pending = []  # (cost_ns, emit_fn) heavy V-work pieces

            def flush(budget):
                while pending and budget > 0:
                    cost, emit = pending.pop(0)
                    emit()
                    budget -= cost

            for t in range(NT):
                heavy = t in HEAVY
                if heavy and acc is None:
                    st = apool.tile([P, M], f16, tag="acc")  # first heavy: st==acc
                else:
                    st = spool.tile([P, M], f16, tag="st")
                for s in range(NSLAB):
                    ps = ppool.tile([P, SLAB], f32, tag="ps")
                    for j in range(SLAB // MM):
                        col = s * SLAB + j * MM
                        g = 32 * (j % NG)
                        nc.tensor.matmul(
                            ps[:, j * MM : (j + 1) * MM],
                            paug_sb[g : g + K, t * P : (t + 1) * P],
                            taug_sb[g : g + K, col : col + MM],
                            start=True,
                            stop=True,
                            tile_position=(g, 0),
                        )
                    dst = st[:, s * SLAB : (s + 1) * SLAB]
                    if s >= NSLAB - N_VCAST[t]:
                        nc.vector.tensor_copy(dst, ps[:])
                    else:
                        nc.scalar.copy(dst, ps[:])
                    # ship light stages per-slab near the end so the tail
                    # overlaps the final converts (whole-tile DMAs otherwise:
                    # small strided transfers waste DMA bandwidth)
                    if not heavy and t >= NT - 2:
                        nc.sync.dma_start(st_out[li, :, s * SLAB : (s + 1) * SLAB], dst)
                if not heavy:
                    if t < NT - 2:
                        nc.sync.dma_start(st_out[li], st[:])
                    li += 1
                else:
                    na, pieces = _heavy_pieces(
                        nc, apool, zpool, st, acc, hi, f3_out, bwd_out
                    )
                    pending.extend(pieces)
                    acc = na
                    hi += 1
                flush(_TILE_V_BUDGET)
            flush(1 << 30)
    nc.compile()
    return nc


def _heavy_pieces(nc, apool, zpool, st, acc, hi, f3_out, bwd_out):
    """Deferred VectorE pieces for a heavy tile: the backward running-max
    TT (two halves) and the forward fold tree, each ~0.7-2.3us."""
    f16 = mybir.dt.float16
    alu_max = mybir.AluOpType.max
    last = hi == len(HEAVY) - 1
    H = M // 2
    if acc is None:
        na = st  # first heavy tile: converts already wrote the accumulator
    else:
        na = apool.tile([P, M], f16, tag="acc")

    pieces = []
    if acc is not None:
        def tt_half(h):
            def emit():
                sl = slice(h * H, (h + 1) * H)
                nc.vector.tensor_tensor(na[:, sl], acc[:, sl], st[:, sl], op=alu_max)
                if last:
                    nc.scalar.dma_start(bwd_out[:, sl], na[:, sl])
            return emit

        pieces += [(2320, tt_half(0)), (2320, tt_half(1))]

    f1 = zpool.tile([P, M // 2], f16, tag="f1")
    f2 = zpool.tile([P, M // 4], f16, tag="f2")
    f3 = zpool.tile([P, M // 8], f16, tag="f3")
    Q = M // 4

    def f1_half(h):
        def emit():
            nc.vector.tensor_tensor(
                f1[:, h * Q : (h + 1) * Q],
                st[:, h * Q : (h + 1) * Q],
                st[:, H + h * Q : H + (h + 1) * Q],
                op=alu_max,
            )
        return emit

    def f23():
        nc.vector.tensor_tensor(f2[:], f1[:, : M // 4], f1[:, M // 4 :], op=alu_max)
        nc.vector.tensor_tensor(f3[:], f2[:, : M // 8], f2[:, M // 8 :], op=alu_max)
        nc.sync.dma_start(f3_out[hi], f3[:])

    pieces += [(1260, f1_half(0)), (1260, f1_half(1)), (1950, f23)]
    return na, pieces


def _get_nc():
    global _cached_nc
    if _cached_nc is None:
        _cached_nc = _build_nc()
    return _cached_nc


def _split16(x):
    """x (f32) -> (hi, lo) fp16 pair with hi + lo ~= x."""
    hi = x.astype(np.float16)
    lo = (x - hi.astype(np.float32)).astype(np.float16)
    return hi, lo


def _replicate_strips(aug):
    """[K, X] -> [128, X] with the data at partition offsets 0/32/64/96."""
    out = np.zeros((P, aug.shape[1]), np.float16)
    for g in range(NG):
        out[32 * g : 32 * g + K] = aug
    return out


def _make_in_maps(pred, target):
    in_maps = []
    for c in range(N_CORES):
        b, h = divmod(c, 2)
        p = pred[b, h * NH : (h + 1) * NH]  # [4096, 3]
        t = target[b]  # [8192, 3]
        pn = -0.5 * (p * p).sum(-1, dtype=np.float32) - 0.5 * DELTA
        tn = -0.5 * (t * t).sum(-1, dtype=np.float32) - 0.5 * DELTA
        ph, pl = _split16(p.T)
        th, tl = _split16(t.T)
        pnh, pnl = _split16(pn)
        tnh, tnl = _split16(tn)
        paug = np.zeros((K, NH), np.float16)
        taug = np.zeros((K, M), np.float16)
        # p.t = ph.th + pl.th + ph.tl ; norms via ones-rows
        paug[0:3] = ph
        paug[3:6] = pl
        paug[6:9] = ph
        paug[9] = pnh
        paug[10] = pnl
        paug[11] = 1.0
        paug[12] = 1.0
        taug[0:3] = th
        taug[3:6] = th
        taug[6:9] = tl
        taug[9] = 1.0
        taug[10] = 1.0
        taug[11] = tnh
        taug[12] = tnl
        prep = _replicate_strips(paug)
        trep = _replicate_strips(taug)
        in_maps.append({
            "pfirst": np.ascontiguousarray(prep[:, :PFIRST]),
            "prest": np.ascontiguousarray(prep[:, PFIRST:]),
            "tfirst": np.ascontiguousarray(trep[:, :TFIRST]),
            "trest": np.ascontiguousarray(trep[:, TFIRST:]),
        })
    return in_maps


def _negmax_bits(u16, axis):
    """Float max of strictly-negative fp16 values stored as uint16 bits:
    more-negative floats have larger bit patterns, so float max == bits min."""
    return np.ascontiguousarray(u16.min(axis=axis)).view(np.float16)


def _reduce_outputs(results):
    total = 0.0
    for b in range(B):
        fwd_rows = []
        bwd_parts = []
        for h in range(2):
            r = results[2 * b + h]
            st = np.asarray(r["st_out"]).view(np.uint16)  # [24, 128, 8192]
            f3 = np.asarray(r["f3_out"]).view(np.uint16)  # [8, 128, 1024]
            bw = np.asarray(r["bwd_out"]).view(np.uint16)  # [128, 8192]
            # forward: per-row max for every pred row, in original tile order
            fwd_light = _negmax_bits(st, axis=2).astype(np.float64)  # [24, 128]
            fwd_heavy = _negmax_bits(f3, axis=2).astype(np.float64)  # [8, 128]
            fwd = np.empty((NT, P))
            fwd[list(LIGHT)] = fwd_light
            fwd[list(HEAVY)] = fwd_heavy
            fwd_rows.append(fwd.reshape(-1))  # row order n = t*128 + p
            # backward: light tiles' columns from the shipped stages,
            # heavy tiles' columns from the on-chip accumulator
            bl = st.min(axis=(0, 1))
            bh = bw.min(axis=0)
            bwd_parts.append(
                np.minimum(bl, bh).view(np.float16).astype(np.float64)
            )
        fwd_max = np.concatenate(fwd_rows)  # [8192]
        bwd_max = np.maximum(bwd_parts[0], bwd_parts[1])  # [8192]
        fwd_sq = np.maximum(-2.0 * (fwd_max + DELTA), EPS)
        bwd_sq = np.maximum(-2.0 * (bwd_max + DELTA), EPS)
        total += np.sqrt(fwd_sq).sum() + np.sqrt(bwd_sq).sum()
    return np.asarray(total / B, dtype=np.float32)


def kernel(pred, target):
    pred = np.ascontiguousarray(np.asarray(pred, dtype=np.float32))
    target = np.ascontiguousarray(np.asarray(target, dtype=np.float32))
    assert pred.shape == (B, N, D) and target.shape == (B, M, D)
    nc = _get_nc()
    in_maps = _make_in_maps(pred, target)
    res = run_bass_kernel_spmd(nc, in_maps, list(range(N_CORES)))
    return _reduce_outputs(res.results)


# revision 27
# speedup vs baseline: 1.1709x; 1.0180x over previous
"""Chamfer distance (weighted, fwd+bwd, mean reduction) on 8 TRN2 NeuronCores.

Math: for pred P[b] (N=8192 x 3) and target T[b] (M=8192 x 3),
  sq(n, m) = |p_n - t_m|^2 = -2 * (p_n . t_m - |p_n|^2/2 - |t_m|^2/2)
One augmented matmul produces out(n, m) = p.t - |p|^2/2 - |t|^2/2 - DELTA
(strictly < 0); then min_m sq = -2 * (max_m out + DELTA) (sqrt is monotone,
applied on host).

The matmul runs in fp16 at full PE rate with a hi/lo split-precision expansion
that recovers fp32-level accuracy:
  p.t = ph.th + pl.th + ph.tl   (pl.tl ~ 2^-22, dropped)
K = 3*3 + 2 + 2 = 13 contraction rows; PE cost is K-independent.

Engine budget (per core, 32M distance-matrix elements): the f32 PSUM ->
fp16 SBUF convert must run on ScalarE/VectorE at 1 elem/lane/cycle (TRN2
matmul cannot write 16-bit PSUM), and pairwise max runs only on VectorE
(fp16 tensor_tensor, 2 elem/lane/cycle).  DMA bandwidth (~330 GB/s) is a
third, independent resource, so tiles come in two flavors tuned so that
ScalarE, VectorE and DMA all finish together:
 - light tiles (24): convert + DMA the full 2MB fp16 stage to DRAM; the
   host does both the forward (row) and backward (column) reductions for
   these rows via a uint16-bits min (exact: stage values are negative).
 - heavy tiles (8, t%4==1): VectorE also does the backward running max
   (8-tile acc chain) and a 3-level forward fold tree; only the 256KB f3
   partials go to DRAM.
Every tile's 4 PSUM slabs are converted by a mix of ScalarE activations
and VectorE casts (42 casts total balances the engines), and heavy-tile
VectorE work is chopped into ~1.2-2.3us pieces drained through a budget
scheduler so neither engine ever idles behind a long in-order burst.

Sharding: batch b -> core pair (2b, 2b+1); each core takes half the pred rows
(4096) and all 8192 targets. Backward partial maxes are combined on host.
"""

import numpy as np

import concourse.bacc as bacc
import concourse.mybir as mybir
import concourse.tile as tile
from concourse.bass_utils import run_bass_kernel_spmd

B = 4
N = 8192  # pred points per batch
M = 8192  # target points per batch
D = 3
K = 13  # augmented contraction dim (split precision)
NH = N // 2  # pred rows per core
P = 128  # partitions
NT = NH // P  # pred tiles per core (32)
SLAB = 2048  # psum slab width (4 banks)
NSLAB = M // SLAB  # 4
MM = 512  # matmul free dim (1 psum bank of f32)
NG = 4  # PE row-group strips at partitions 0/32/64/96
TFIRST = SLAB  # operand prefix that gates the first slab's matmuls
PFIRST = P
N_CORES = 8
EPS = 1e-12
DELTA = 2.0**-10  # subtracted inside the matmul so every stage value is < 0

HEAVY = tuple(t for t in range(NT) if t % 4 == 1)  # on-chip fwd+bwd tiles
LIGHT = tuple(t for t in range(NT) if t not in HEAVY)
# VectorE casts this many PSUM slabs per tile (42 total balances ScalarE)
N_VCAST = {t: (2 if t % 3 == 2 else 1) for t in range(NT)}

# measured per-op VectorE costs (ns) for the piece scheduler
_COST_CAST = 2290
_TILE_V_BUDGET = 5300  # target VectorE ns per tile slot

_cached_nc = None


def _build_nc():
    f32 = mybir.dt.float32
    f16 = mybir.dt.float16
    alu_max = mybir.AluOpType.max

    nc = bacc.Bacc("TRN2", target_bir_lowering=False, debug=False)
    # Operands arrive pre-replicated into the 4 row-group strips (rows
    # 32g..32g+12 hold the data, the rest are zero), split into a small
    # first chunk (gates the first matmuls) and the bulk.
    pfirst = nc.dram_tensor("pfirst", [P, PFIRST], f16, kind="ExternalInput")
    prest = nc.dram_tensor("prest", [P, NH - PFIRST], f16, kind="ExternalInput")
    tfirst = nc.dram_tensor("tfirst", [P, TFIRST], f16, kind="ExternalInput")
    trest = nc.dram_tensor("trest", [P, M - TFIRST], f16, kind="ExternalInput")
    # st_out[i] = full fp16 stage of the i-th light tile (host reduces)
    st_out = nc.dram_tensor("st_out", [len(LIGHT), P, M], f16, kind="ExternalOutput")
    # f3_out[i] = 1024-wide forward partials of the i-th heavy tile
    f3_out = nc.dram_tensor("f3_out", [len(HEAVY), P, M // 8], f16, kind="ExternalOutput")
    # bwd_out[p, m] = max over the heavy tiles' pred rows congruent to p
    bwd_out = nc.dram_tensor("bwd_out", [P, M], f16, kind="ExternalOutput")

    with tile.TileContext(nc) as tc:
        with (
            tc.tile_pool(name="const", bufs=1) as cpool,
            tc.tile_pool(name="stage", bufs=6) as spool,
            tc.tile_pool(name="accp", bufs=2) as apool,
            tc.tile_pool(name="fold", bufs=2) as zpool,
            tc.tile_pool(name="psum", bufs=2, space="PSUM") as ppool,
        ):
            taug_sb = cpool.tile([P, M], f16)
            paug_sb = cpool.tile([P, NH], f16)
            nc.sync.dma_start(paug_sb[:, :PFIRST], pfirst[:])
            nc.sync.dma_start(taug_sb[:, :TFIRST], tfirst[:])
            nc.sync.dma_start(taug_sb[:, TFIRST:], trest[:])
            nc.scalar.dma_start(paug_sb[:, PFIRST:], prest[:])

            acc = None
            li = 0
            hi = 0
            pending = []  # (cost_ns, emit_fn) heavy V-work pieces

            def flush(budget):
                while pending and budget > 0:
                    cost, emit = pending.pop(0)
                    emit()
                    budget -= cost

            for t in range(NT):
                heavy = t in HEAVY
                if heavy and acc is None:
                    st = apool.tile([P, M], f16, tag="acc")  # first heavy: st==acc
                else:
                    st = spool.tile([P, M], f16, tag="st")
                nvc = N_VCAST[t]
                for s in range(NSLAB):
                    ps = ppool.tile([P, SLAB], f32, tag="ps")
                    for j in range(SLAB // MM):
                        col = s * SLAB + j * MM
                        g = 32 * (j % NG)
                        nc.tensor.matmul(
                            ps[:, j * MM : (j + 1) * MM],
                            paug_sb[g : g + K, t * P : (t + 1) * P],
                            taug_sb[g : g + K, col : col + MM],
                            start=True,
                            stop=True,
                            tile_position=(g, 0),
                        )
                    dst = st[:, s * SLAB : (s + 1) * SLAB]
                    if s >= NSLAB - nvc:
                        nc.vector.tensor_copy(dst, ps[:])
                    else:
                        nc.scalar.copy(dst, ps[:])
                    # ship light stages per-slab near the end so the tail
                    # overlaps the final converts (whole-tile DMAs otherwise:
                    # small strided transfers waste DMA bandwidth)
                    if not heavy and t >= NT - 2:
                        nc.sync.dma_start(st_out[li, :, s * SLAB : (s + 1) * SLAB], dst)
                if not heavy:
                    if t < NT - 2:
                        nc.sync.dma_start(st_out[li], st[:])
                    li += 1
                else:
                    na, pieces = _heavy_pieces(
                        nc, apool, zpool, st, acc, hi, f3_out, bwd_out
                    )
                    pending.extend(pieces)
                    acc = na
                    hi += 1
                flush(_TILE_V_BUDGET - nvc * _COST_CAST)
            flush(1 << 30)
    nc.compile()
    return nc


def _heavy_pieces(nc, apool, zpool, st, acc, hi, f3_out, bwd_out):
    """Deferred VectorE pieces for a heavy tile: the backward running-max
    TT (two halves) and the forward fold tree, each ~0.7-2.3us."""
    f16 = mybir.dt.float16
    alu_max = mybir.AluOpType.max
    last = hi == len(HEAVY) - 1
    H = M // 2
    if acc is None:
        na = st  # first heavy tile: converts already wrote the accumulator
    else:
        na = apool.tile([P, M], f16, tag="acc")

    pieces = []
    if acc is not None:
        def tt_half(h):
            def emit():
                sl = slice(h * H, (h + 1) * H)
                nc.vector.tensor_tensor(na[:, sl], acc[:, sl], st[:, sl], op=alu_max)
                if last:
                    nc.scalar.dma_start(bwd_out[:, sl], na[:, sl])
            return emit

        pieces += [(2320, tt_half(0)), (2320, tt_half(1))]

    f1 = zpool.tile([P, M // 2], f16, tag="f1")
    f2 = zpool.tile([P, M // 4], f16, tag="f2")
    f3 = zpool.tile([P, M // 8], f16, tag="f3")
    Q = M // 4

    def f1_half(h):
        def emit():
            nc.vector.tensor_tensor(
                f1[:, h * Q : (h + 1) * Q],
                st[:, h * Q : (h + 1) * Q],
                st[:, H + h * Q : H + (h + 1) * Q],
                op=alu_max,
            )
        return emit

    def f23():
        nc.vector.tensor_tensor(f2[:], f1[:, : M // 4], f1[:, M // 4 :], op=alu_max)
        nc.vector.tensor_tensor(f3[:], f2[:, : M // 8], f2[:, M // 8 :], op=alu_max)
        nc.sync.dma_start(f3_out[hi], f3[:])

    pieces += [(1260, f1_half(0)), (1260, f1_half(1)), (1950, f23)]
    return na, pieces


def _get_nc():
    global _cached_nc
    if _cached_nc is None:
        _cached_nc = _build_nc()
    return _cached_nc


def _split16(x):
    """x (f32) -> (hi, lo) fp16 pair with hi + lo ~= x."""
    hi = x.astype(np.float16)
    lo = (x - hi.astype(np.float32)).astype(np.float16)
    return hi, lo


def _replicate_strips(aug):
    """[K, X] -> [128, X] with the data at partition offsets 0/32/64/96."""
    out = np.zeros((P, aug.shape[1]), np.float16)
    for g in range(NG):
        out[32 * g : 32 * g + K] = aug
    return out


def _make_in_maps(pred, target):
    in_maps = []
    for c in range(N_CORES):
        b, h = divmod(c, 2)
        p = pred[b, h * NH : (h + 1) * NH]  # [4096, 3]
        t = target[b]  # [8192, 3]
        pn = -0.5 * (p * p).sum(-1, dtype=np.float32) - 0.5 * DELTA
        tn = -0.5 * (t * t).sum(-1, dtype=np.float32) - 0.5 * DELTA
        ph, pl = _split16(p.T)
        th, tl = _split16(t.T)
        pnh, pnl = _split16(pn)
        tnh, tnl = _split16(tn)
        paug = np.zeros((K, NH), np.float16)
        taug = np.zeros((K, M), np.float16)
        # p.t = ph.th + pl.th + ph.tl ; norms via ones-rows
        paug[0:3] = ph
        paug[3:6] = pl
        paug[6:9] = ph
        paug[9] = pnh
        paug[10] = pnl
        paug[11] = 1.0
        paug[12] = 1.0
        taug[0:3] = th
        taug[3:6] = th
        taug[6:9] = tl
        taug[9] = 1.0
        taug[10] = 1.0
        taug[11] = tnh
        taug[12] = tnl
        prep = _replicate_strips(paug)
        trep = _replicate_strips(taug)
        in_maps.append({
            "pfirst": np.ascontiguousarray(prep[:, :PFIRST]),
            "prest": np.ascontiguousarray(prep[:, PFIRST:]),
            "tfirst": np.ascontiguousarray(trep[:, :TFIRST]),
            "trest": np.ascontiguousarray(trep[:, TFIRST:]),
        })
    return in_maps


def _negmax_bits(u16, axis):
    """Float max of strictly-negative fp16 values stored as uint16 bits:
    more-negative floats have larger bit patterns, so float max == bits min."""
    return np.ascontiguousarray(u16.min(axis=axis)).view(np.float16)


def _reduce_outputs(results):
    total = 0.0
    for b in range(B):
        fwd_rows = []
        bwd_parts = []
        for h in range(2):
            r = results[2 * b + h]
            st = np.asarray(r["st_out"]).view(np.uint16)  # [24, 128, 8192]
            f3 = np.asarray(r["f3_out"]).view(np.uint16)  # [8, 128, 1024]
            bw = np.asarray(r["bwd_out"]).view(np.uint16)  # [128, 8192]
            # forward: per-row max for every pred row, in original tile order
            fwd_light = _negmax_bits(st, axis=2).astype(np.float64)  # [24, 128]
            fwd_heavy = _negmax_bits(f3, axis=2).astype(np.float64)  # [8, 128]
            fwd = np.empty((NT, P))
            fwd[list(LIGHT)] = fwd_light
            fwd[list(HEAVY)] = fwd_heavy
            fwd_rows.append(fwd.reshape(-1))  # row order n = t*128 + p
            # backward: light tiles' columns from the shipped stages,
            # heavy tiles' columns from the on-chip accumulator
            bl = st.min(axis=(0, 1))
            bh = bw.min(axis=0)
            bwd_parts.append(
                np.minimum(bl, bh).view(np.float16).astype(np.float64)
            )
        fwd_max = np.concatenate(fwd_rows)  # [8192]
        bwd_max = np.maximum(bwd_parts[0], bwd_parts[1])  # [8192]
        fwd_sq = np.maximum(-2.0 * (fwd_max + DELTA), EPS)
        bwd_sq = np.maximum(-2.0 * (bwd_max + DELTA), EPS)
        total += np.sqrt(fwd_sq).sum() + np.sqrt(bwd_sq).sum()
    return np.asarray(total / B, dtype=np.float32)


def kernel(pred, target):
    pred = np.ascontiguousarray(np.asarray(pred, dtype=np.float32))
    target = np.ascontiguousarray(np.asarray(target, dtype=np.float32))
    assert pred.shape == (B, N, D) and target.shape == (B, M, D)
    nc = _get_nc()
    in_maps = _make_in_maps(pred, target)
    res = run_bass_kernel_spmd(nc, in_maps, list(range(N_CORES)))
    return _reduce_outputs(res.results)


# revision 32
# speedup vs baseline: 1.1907x; 1.0169x over previous
"""Chamfer distance (weighted, fwd+bwd, mean reduction) on 8 TRN2 NeuronCores.

Math: for pred P[b] (N=8192 x 3) and target T[b] (M=8192 x 3),
  sq(n, m) = |p_n - t_m|^2 = -2 * (p_n . t_m - |p_n|^2/2 - |t_m|^2/2)
One augmented matmul produces out(n, m) = p.t - |p|^2/2 - |t|^2/2 - DELTA
(strictly < 0); then min_m sq = -2 * (max_m out + DELTA) (sqrt is monotone,
applied on host).

The matmul runs in fp16 at full PE rate with a hi/lo split-precision expansion
that recovers fp32-level accuracy:
  p.t = ph.th + pl.th + ph.tl   (pl.tl ~ 2^-22, dropped)
K = 3*3 + 2 + 2 = 13 contraction rows; PE cost is K-independent.

Engine budget (per core, 32M distance-matrix elements): the f32 PSUM ->
fp16 SBUF convert must run on ScalarE/VectorE at 1 elem/lane/cycle (TRN2
matmul cannot write 16-bit PSUM), and pairwise max runs only on VectorE
(fp16 tensor_tensor, 2 elem/lane/cycle).  DMA bandwidth (~330 GB/s) is a
third, independent resource, so tiles come in two flavors tuned so that
ScalarE, VectorE and DMA all finish together:
 - light tiles (24): convert + DMA the full 2MB fp16 stage to DRAM; the
   host does both the forward (row) and backward (column) reductions for
   these rows via a uint16-bits min (exact: stage values are negative).
 - heavy tiles (8, t%4==1): VectorE also does the backward running max
   (8-tile acc chain) and a 3-level forward fold tree; only the 256KB f3
   partials go to DRAM.
Every tile's 4 PSUM slabs are converted by a mix of ScalarE activations
and VectorE casts (42 casts total balances the engines), and heavy-tile
VectorE work is chopped into ~1.2-2.3us pieces drained through a budget
scheduler so neither engine ever idles behind a long in-order burst.

Sharding: batch b -> core pair (2b, 2b+1); each core takes half the pred rows
(4096) and all 8192 targets. Backward partial maxes are combined on host.
"""

import numpy as np

import concourse.bacc as bacc
import concourse.mybir as mybir
import concourse.tile as tile
from concourse.bass_utils import run_bass_kernel_spmd

B = 4
N = 8192  # pred points per batch
M = 8192  # target points per batch
D = 3
K = 13  # augmented contraction dim (split precision)
NH = N // 2  # pred rows per core
P = 128  # partitions
NT = NH // P  # pred tiles per core (32)
SLAB = 2048  # psum slab width (4 banks)
NSLAB = M // SLAB  # 4
MM = 512  # matmul free dim (1 psum bank of f32)
NG = 4  # PE row-group strips at partitions 0/32/64/96
TFIRST = SLAB  # operand prefix that gates the first slab's matmuls
PFIRST = P
N_CORES = 8
EPS = 1e-12
DELTA = 2.0**-10  # subtracted inside the matmul so every stage value is < 0

HEAVY = tuple(t for t in range(NT) if t % 4 == 1)  # on-chip fwd+bwd tiles
LIGHT = tuple(t for t in range(NT) if t not in HEAVY)
# VectorE casts this many PSUM slabs per tile (42 total balances ScalarE)
N_VCAST = {t: (2 if t % 3 == 2 else 1) for t in range(NT)}

# measured per-op VectorE costs (ns) for the piece scheduler
_COST_CAST = 2290
_TILE_V_BUDGET = 5300  # target VectorE ns per tile slot

_cached_nc = None


def _build_nc():
    f32 = mybir.dt.float32
    f16 = mybir.dt.float16
    alu_max = mybir.AluOpType.max

    nc = bacc.Bacc("TRN2", target_bir_lowering=False, debug=False)
    # Operands arrive pre-replicated into the 4 row-group strips (rows
    # 32g..32g+12 hold the data, the rest are zero), split into a small
    # first chunk (gates the first matmuls) and the bulk.
    pfirst = nc.dram_tensor("pfirst", [P, PFIRST], f16, kind="ExternalInput")
    prest = nc.dram_tensor("prest", [P, NH - PFIRST], f16, kind="ExternalInput")
    tfirst = nc.dram_tensor("tfirst", [P, TFIRST], f16, kind="ExternalInput")
    trest = nc.dram_tensor("trest", [P, M - TFIRST], f16, kind="ExternalInput")
    # st_out[i] = full fp16 stage of the i-th light tile (host reduces)
    st_out = nc.dram_tensor(
        "st_out", [len(LIGHT) - 2, P, M], f16, kind="ExternalOutput"
    )
    # last two light tiles ship per-slab into a slab-major (contiguous
    # 512KB per transfer) layout so the tail drains right after the
    # final converts instead of queuing 2MB strided transfers
    st_tail = nc.dram_tensor("st_tail", [2, NSLAB, P, SLAB], f16, kind="ExternalOutput")
    # f3_out[i] = 1024-wide forward partials of the i-th heavy tile
    f3_out = nc.dram_tensor("f3_out", [len(HEAVY), P, M // 8], f16, kind="ExternalOutput")
    # bwd_out[p, m] = max over the heavy tiles' pred rows congruent to p
    bwd_out = nc.dram_tensor("bwd_out", [P, M], f16, kind="ExternalOutput")

    with tile.TileContext(nc) as tc:
        with (
            tc.tile_pool(name="const", bufs=1) as cpool,
            tc.tile_pool(name="stage", bufs=6) as spool,
            tc.tile_pool(name="accp", bufs=2) as apool,
            tc.tile_pool(name="fold", bufs=2) as zpool,
            tc.tile_pool(name="psum", bufs=2, space="PSUM") as ppool,
        ):
            taug_sb = cpool.tile([P, M], f16)
            paug_sb = cpool.tile([P, NH], f16)
            nc.sync.dma_start(paug_sb[:, :PFIRST], pfirst[:])
            # per-matmul chunks so the first matmul isn't gated on the
            # whole first slab's operands
            for j in range(TFIRST // MM):
                nc.sync.dma_start(
                    taug_sb[:, j * MM : (j + 1) * MM],
                    tfirst[:, j * MM : (j + 1) * MM],
                )
            nc.sync.dma_start(taug_sb[:, TFIRST:], trest[:])
            nc.scalar.dma_start(paug_sb[:, PFIRST:], prest[:])

            acc = None
            li = 0
            hi = 0
            pending = []  # (cost_ns, emit_fn) heavy V-work pieces

            def flush(budget):
                while pending and budget > 0:
                    cost, emit = pending.pop(0)
                    emit()
                    budget -= cost

            for t in range(NT):
                heavy = t in HEAVY
                if heavy and acc is None:
                    st = apool.tile([P, M], f16, tag="acc")  # first heavy: st==acc
                else:
                    st = spool.tile([P, M], f16, tag="st")
                nvc = N_VCAST[t]
                for s in range(NSLAB):
                    ps = ppool.tile([P, SLAB], f32, tag="ps")
                    for j in range(SLAB // MM):
                        col = s * SLAB + j * MM
                        g = 32 * (j % NG)
                        nc.tensor.matmul(
                            ps[:, j * MM : (j + 1) * MM],
                            paug_sb[g : g + K, t * P : (t + 1) * P],
                            taug_sb[g : g + K, col : col + MM],
                            start=True,
                            stop=True,
                            tile_position=(g, 0),
                        )
                    dst = st[:, s * SLAB : (s + 1) * SLAB]
                    if s >= NSLAB - nvc:
                        nc.vector.tensor_copy(dst, ps[:])
                    else:
                        nc.scalar.copy(dst, ps[:])
                    # ship light stages per-slab near the end so the tail
                    # overlaps the final converts (contiguous slab-major dst)
                    if not heavy and t >= NT - 2:
                        nc.sync.dma_start(st_tail[t - (NT - 2), s], dst)
                if not heavy:
                    if t < NT - 2:
                        nc.sync.dma_start(st_out[li], st[:])
                    li += 1
                else:
                    na, pieces = _heavy_pieces(
                        nc, apool, zpool, st, acc, hi, f3_out, bwd_out
                    )
                    pending.extend(pieces)
                    acc = na
                    hi += 1
                flush(_TILE_V_BUDGET - nvc * _COST_CAST)
            flush(1 << 30)
    nc.compile()
    return nc


def _heavy_pieces(nc, apool, zpool, st, acc, hi, f3_out, bwd_out):
    """Deferred VectorE pieces for a heavy tile: the backward running-max
    TT (two halves) and the forward fold tree, each ~0.7-2.3us."""
    f16 = mybir.dt.float16
    alu_max = mybir.AluOpType.max
    last = hi == len(HEAVY) - 1
    H = M // 2
    if acc is None:
        na = st  # first heavy tile: converts already wrote the accumulator
    else:
        na = apool.tile([P, M], f16, tag="acc")

    pieces = []
    if acc is not None:
        def tt_half(h):
            def emit():
                sl = slice(h * H, (h + 1) * H)
                nc.vector.tensor_tensor(na[:, sl], acc[:, sl], st[:, sl], op=alu_max)
                if last:
                    nc.scalar.dma_start(bwd_out[:, sl], na[:, sl])
            return emit

        pieces += [(2320, tt_half(0)), (2320, tt_half(1))]

    f1 = zpool.tile([P, M // 2], f16, tag="f1")
    f2 = zpool.tile([P, M // 4], f16, tag="f2")
    f3 = zpool.tile([P, M // 8], f16, tag="f3")
    Q = M // 4

    def f1_half(h):
        def emit():
            nc.vector.tensor_tensor(
                f1[:, h * Q : (h + 1) * Q],
                st[:, h * Q : (h + 1) * Q],
                st[:, H + h * Q : H + (h + 1) * Q],
                op=alu_max,
            )
        return emit

    def f23():
        nc.vector.tensor_tensor(f2[:], f1[:, : M // 4], f1[:, M // 4 :], op=alu_max)
        nc.vector.tensor_tensor(f3[:], f2[:, : M // 8], f2[:, M // 8 :], op=alu_max)
        nc.sync.dma_start(f3_out[hi], f3[:])

    pieces += [(1260, f1_half(0)), (1260, f1_half(1)), (1950, f23)]
    return na, pieces


def _get_nc():
    global _cached_nc
    if _cached_nc is None:
        _cached_nc = _build_nc()
    return _cached_nc


def _split16(x):
    """x (f32) -> (hi, lo) fp16 pair with hi + lo ~= x."""
    hi = x.astype(np.float16)
    lo = (x - hi.astype(np.float32)).astype(np.float16)
    return hi, lo


def _replicate_strips(aug):
    """[K, X] -> [128, X] with the data at partition offsets 0/32/64/96."""
    out = np.zeros((P, aug.shape[1]), np.float16)
    for g in range(NG):
        out[32 * g : 32 * g + K] = aug
    return out


def _make_in_maps(pred, target):
    in_maps = []
    for c in range(N_CORES):
        b, h = divmod(c, 2)
        p = pred[b, h * NH : (h + 1) * NH]  # [4096, 3]
        t = target[b]  # [8192, 3]
        pn = -0.5 * (p * p).sum(-1, dtype=np.float32) - 0.5 * DELTA
        tn = -0.5 * (t * t).sum(-1, dtype=np.float32) - 0.5 * DELTA
        ph, pl = _split16(p.T)
        th, tl = _split16(t.T)
        pnh, pnl = _split16(pn)
        tnh, tnl = _split16(tn)
        paug = np.zeros((K, NH), np.float16)
        taug = np.zeros((K, M), np.float16)
        # p.t = ph.th + pl.th + ph.tl ; norms via ones-rows
        paug[0:3] = ph
        paug[3:6] = pl
        paug[6:9] = ph
        paug[9] = pnh
        paug[10] = pnl
        paug[11] = 1.0
        paug[12] = 1.0
        taug[0:3] = th
        taug[3:6] = th
        taug[6:9] = tl
        taug[9] = 1.0
        taug[10] = 1.0
        taug[11] = tnh
        taug[12] = tnl
        prep = _replicate_strips(paug)
        trep = _replicate_strips(taug)
        in_maps.append({
            "pfirst": np.ascontiguousarray(prep[:, :PFIRST]),
            "prest": np.ascontiguousarray(prep[:, PFIRST:]),
            "tfirst": np.ascontiguousarray(trep[:, :TFIRST]),
            "trest": np.ascontiguousarray(trep[:, TFIRST:]),
        })
    return in_maps


def _negmax_bits(u16, axis):
    """Float max of strictly-negative fp16 values stored as uint16 bits:
    more-negative floats have larger bit patterns, so float max == bits min."""
    return np.ascontiguousarray(u16.min(axis=axis)).view(np.float16)


def _reduce_outputs(results):
    total = 0.0
    for b in range(B):
        fwd_rows = []
        bwd_parts = []
        for h in range(2):
            r = results[2 * b + h]
            st = np.asarray(r["st_out"]).view(np.uint16)  # [22, 128, 8192]
            tail = np.asarray(r["st_tail"]).view(np.uint16)  # [2, 4, 128, 2048]
            f3 = np.asarray(r["f3_out"]).view(np.uint16)  # [8, 128, 1024]
            bw = np.asarray(r["bwd_out"]).view(np.uint16)  # [128, 8192]
            # forward: per-row max for every pred row, in original tile order
            fwd_light = np.concatenate(
                [
                    _negmax_bits(st, axis=2),  # [22, 128]
                    _negmax_bits(
                        tail.transpose(0, 2, 1, 3).reshape(2, P, M), axis=2
                    ),  # [2, 128]
                ]
            ).astype(np.float64)
            fwd_heavy = _negmax_bits(f3, axis=2).astype(np.float64)  # [8, 128]
            fwd = np.empty((NT, P))
            fwd[list(LIGHT)] = fwd_light
            fwd[list(HEAVY)] = fwd_heavy
            fwd_rows.append(fwd.reshape(-1))  # row order n = t*128 + p
            # backward: light tiles' columns from the shipped stages,
            # heavy tiles' columns from the on-chip accumulator
            bl = st.min(axis=(0, 1))
            bt = tail.min(axis=(0, 2)).reshape(-1)  # [4*2048] column order
            bh = bw.min(axis=0)
            bwd_parts.append(
                np.minimum(np.minimum(bl, bt), bh).view(np.float16).astype(np.float64)
            )
        fwd_max = np.concatenate(fwd_rows)  # [8192]
        bwd_max = np.maximum(bwd_parts[0], bwd_parts[1])  # [8192]
        fwd_sq = np.maximum(-2.0 * (fwd_max + DELTA), EPS)
        bwd_sq = np.maximum(-2.0 * (bwd_max + DELTA), EPS)
        total += np.sqrt(fwd_sq).sum() + np.sqrt(bwd_sq).sum()
    return np.asarray(total / B, dtype=np.float32)


def kernel(pred, target):
    pred = np.ascontiguousarray(np.asarray(pred, dtype=np.float32))
    target = np.ascontiguousarray(np.asarray(target, dtype=np.float32))
    assert pred.shape == (B, N, D) and target.shape == (B, M, D)
    nc = _get_nc()
    in_maps = _make_in_maps(pred, target)
    res = run_bass_kernel_spmd(nc, in_maps, list(range(N_CORES)))
    return _reduce_outputs(res.results)
